# revision 83
# baseline (speedup 1.0000x reference)
"""Trainium2 Bass kernel for nn_ContrastiveCriterion.

Reference semantics (per sample b of B=2, N=4096, D=512):
    refer = l2_normalize(emb_point[b][pos_idx[b]])      # [N, D]
    key   = l2_normalize(emb_text[b])                   # [N, D]
    sim   = refer @ key.T                               # [N, N]
    ce_p[i] = logsumexp_j(ls*sim[i,j]) - ls*sim[i,i]
    ce_t[j] = logsumexp_i(ls*sim[i,j]) - ls*sim[j,j]
    loss_b  = mean_i(0.5*(ce_p+ce_t)*dist_norm[b])
    rank_b  = sum_ij relu(sim[i,j] - sim[j,j])
    out = (mean_b loss_b, 0.5 * mean_b rank_b)

Device work is the O(N^2 D) part only: each of 8 cores (= 2 samples x 4
row-chunks of 1024) computes its 1024x4096 slab of sim exactly once (the
baseline computed it twice - rows and transposed rows), and from each PSUM
quarter [128, 1024] derives, fused:
  - rowsums of exp(ls*sim)           (ACT Exp with accum_out)      -> ce_p
  - colsum partials of exp(ls*sim)   (bf16 tile accumulate, adds split
    DVE/Pool, + ones-matmul partition-reduce per column quarter)   -> ce_t
  - rowsums of max(sim, d[j])        (DVE scalar_tensor_tensor with
    accum_out; GPSIMD cannot read PSUM)                            -> rank
    using sum_j relu(sim-d[j]) = sum_j max(sim,d[j]) - sum_j d[j].
The O(N D) prep (l2 norms, gather, diagonal d[j] = <refer_n[j], key_n[j]>,
operand transposes into block-packed layouts) and the final O(N) log/mean
reductions run on the host.

Schedule notes (TimelineSim-driven):
  - jq (column quarter) is the outer loop so the first matmul waits only on
    the slab + key quarter 0 (one merged "slk" DMA; the sim serializes DMA,
    so DMA count on the head path matters).
  - throwaway ones-matmuls warm the PE clock during the DMA head (the cost
    model's pstate ramp would otherwise run the first ~3us at 1.2GHz).
  - each quarter's colsum group-0 pass runs at ti==6 of the same quarter,
    the rest mid-way through the NEXT quarter (acc double-buffered on jq
    parity), so the PE stream never stalls on the accumulate adds.
  - the last quarter's ti7 exp tile feeds the final colsum matmul directly
    (its accumulate add would otherwise sit on the tail critical chain).
"""

import numpy as np
import ml_dtypes

import concourse.bass as bass
import concourse.tile as tile
import concourse.mybir as mybir
from concourse.bass_utils import run_bass_kernel_spmd

B, N, D = 2, 4096, 512
P = 128                 # SBUF partitions
KC = D // P             # 4 contraction chunks
QPER = 4                # cores per sample
CHUNK = N // QPER       # 1024 rows per core
TI = CHUNK // P         # 8 row tiles per core
QW = 1024               # column quarter width (psum tile = 2 banks)
JQ = N // QW            # 4 column quarters
NPART = TI * JQ         # 32 partial columns in the stats outputs

bf16 = mybir.dt.bfloat16
f32 = mybir.dt.float32

# set by kernel() for test harness introspection
LAST_RESULT = None

# walrus codegen for TRN2 CTRL instructions (Drain) accepts a limited number
# of sync-wait slots; Tile's kernel-tail drain can carry one wait per live
# semaphore.  Split any over-limit drain into a chain of drains, each
# carrying at most MAX_DRAIN_WAITS waits (same-engine program order makes
# the chain equivalent to the single multi-wait drain).
MAX_DRAIN_WAITS = 1


def _split_drain_waits(nc: bass.Bass, max_waits: int = MAX_DRAIN_WAITS) -> None:
    for fn in nc.m.functions:
        for bb in fn.blocks:
            insts = list(bb.instructions)
            out, n_extra = [], 0
            for ins in insts:
                cap = max_waits
                si = ins.sync_info
                if si is not None and si.on_wait and len(si.on_wait) > cap:
                    waits = list(si.on_wait)
                    for k in range(0, len(waits) - cap, 1):
                        extra = mybir.InstDrain(
                            name=f"{ins.name}_prewait{k}",
                            ins=[],
                            outs=[],
                        )
                        extra.engine = ins.engine
                        extra.sync_info = mybir.SyncInfo(
                            on_wait=waits[k: k + 1], on_update=[]
                        )
                        out.append(extra)
                        n_extra += 1
                    si.on_wait = waits[len(waits) - cap:]
                out.append(ins)
            if n_extra:
                bb.instructions[:] = out


def build_program(logit_scale: float) -> bass.Bass:
    nc = bass.Bass()

    # fp8 DoubleRow operands.  Each value x is split on the host into
    # x = hi + lo (both float8e4); sim = xh@yh + xh@yl + xl@yh, computed as
    # 6 DoubleRow passes per 512-col psum block (contraction pair s*128+p).
    # Layout of every pack is [P, 2(s), W]: slab blocks at (ti*2+cp)*128
    # (cp = chunk pair), key blocks at (j2*2+cp)*512.
    # slk merges slab hi/lo row-tiles 0-1 (cols 0:512 hi, 512:1024 lo) with
    # key quarter 0 (cols 1024:3072 hi, 3072:5120 lo) -- one DMA covers the
    # whole head (the sim serializes DMAs, so DMA count matters there).
    # slr holds slab row-tiles 2-7 (hi then lo), tx the key quarters 1..3
    # (per quarter: 2048 hi then 2048 lo).
    f8 = mybir.dt.float8e4
    SLK_W = 2 * 512 + 2 * 2048
    SLR_W = 2 * (TI - 2) * 256
    KTQ_W = 2 * 2048
    slk = nc.declare_dram_parameter("slk", [P, 2, SLK_W], f8, isOutput=False)
    slr = nc.declare_dram_parameter("slr", [P, 2, SLR_W], f8, isOutput=False)
    tx = nc.declare_dram_parameter("tx", [P, 2, 3 * KTQ_W], f8, isOutput=False)
    db = nc.declare_dram_parameter("db", [P, N], bf16, isOutput=False)
    out_pr = nc.declare_dram_parameter("out_pr", [P, 2 * NPART], f32,
                                       isOutput=True)
    out_st = nc.declare_dram_parameter("out_st", [1, N], f32, isOutput=True)

    Act = mybir.ActivationFunctionType
    Alu = mybir.AluOpType

    with tile.TileContext(nc) as tc:
        with tc.tile_pool(name="main", bufs=1) as pmain:
            # block-packed operands (layouts mirror slk/tx dram params)
            slk_t = pmain.tile([P, 2, SLK_W], f8, name="slk_t", tag="slk_t")
            slr_t = pmain.tile([P, 2, SLR_W], f8, name="slr_t", tag="slr_t")
            ktq = [None] + [pmain.tile([P, 2, KTQ_W], f8, name=f"ktq{jq}",
                                       tag=f"ktq{jq}") for jq in range(1, JQ)]
            dfq = [pmain.tile([P, QW], bf16, name=f"dfq{jq}", tag=f"dfq{jq}")
                   for jq in range(JQ)]
            ones_t = pmain.tile([P, P], bf16, name="ones_t", tag="ones_t")
            pr_parts = pmain.tile([P, 2 * NPART], f32, name="pr_parts",
                                  tag="pr_parts")
            # exp(ls*sim) accumulators for the column-sum partition-reduce:
            # acc[qg][pb] = sum over 4 row tiles (ti in qg*4..qg*4+4) of the
            # exp tile for the current column quarter.  Double-buffered on jq
            # parity (pb) so quarter jq's colsum matmuls can be deferred into
            # the middle of quarter jq+1 without blocking its direct writes.
            acc = [[pmain.tile([P, QW], bf16, name=f"acc{qg}_{pb}", tag=f"acc{qg}_{pb}")
                    for pb in range(2)] for qg in range(2)]
            st_row = pmain.tile([1, N], f32, name="st_row", tag="st_row")

            nc.vector.memset(ones_t, 1.0)

            # ---- loads: plain contiguous DMAs in consumption order.  The
            # first matmul needs only sl block ti=0 and tx block (jq=0,j2=0).
            nc.sync.dma_start(out=slk_t[:, :, 0:2560], in_=slk[:, :, 0:2560])
            nc.sync.dma_start(out=slk_t[:, :, 2560:], in_=slk[:, :, 2560:])
            nc.sync.dma_start(out=dfq[0], in_=db[:, 0:QW])
            nc.sync.dma_start(out=slr_t, in_=slr[:, :, :])
            for jq in range(1, JQ):
                ks = slice((jq - 1) * KTQ_W, jq * KTQ_W)
                nc.sync.dma_start(out=ktq[jq], in_=tx[:, :, ks])
                nc.sync.dma_start(out=dfq[jq], in_=db[:, jq * QW:(jq + 1) * QW])

            with tc.tile_pool(name="scr", bufs=3) as pscr:
                with tc.tile_pool(name="psA", bufs=1, space="PSUM") as ppa, \
                     tc.tile_pool(name="psB", bufs=1, space="PSUM") as ppb:
                    # ---- column-sum partition-reduce for quarter jq: two
                    # ones-matmul passes per row-tile group, then copy the
                    # broadcast row to SBUF.  Emitted mid-way through quarter
                    # jq+1 so the PE queue never stalls on the acc adds.
                    psts = {}
                    esc_tail = [None, None]

                    def colsum_g0(jq):
                        pb = jq % 2
                        pst = ppb.tile([P, QW], f32, name=f"pst_{jq}", tag="st",
                                       bufs=1)
                        psts[jq] = pst
                        for j2 in range(QW // 512):
                            blk = slice(j2 * 512, (j2 + 1) * 512)
                            nc.tensor.matmul(pst[:, blk], lhsT=ones_t,
                                             rhs=acc[0][pb][:, blk],
                                             start=True, stop=False)

                    def colsum(jq, extra=()):
                        # rhs-outer emission: passes that wait on the last
                        # exp tiles come LAST so they don't block the
                        # independent ones in the in-order PE queue.
                        pb = jq % 2
                        pst = psts.pop(jq)
                        srcs = [acc[1][pb]] + list(extra)
                        for k, e in enumerate(srcs):
                            for j2 in range(QW // 512):
                                blk = slice(j2 * 512, (j2 + 1) * 512)
                                nc.tensor.matmul(pst[:, blk], lhsT=ones_t,
                                                 rhs=e[:, blk],
                                                 start=False,
                                                 stop=(k == len(srcs) - 1))
                        for j2 in range(QW // 512):
                            blk = slice(j2 * 512, (j2 + 1) * 512)
                            qs2 = slice(jq * QW + j2 * 512,
                                        jq * QW + (j2 + 1) * 512)
                            nc.scalar.activation(st_row[0:1, qs2],
                                                 pst[0:1, blk], Act.Copy)
                        qs = slice(jq * QW, (jq + 1) * QW)
                        lo = 2 * jq * TI
                        if jq == JQ - 1:
                            # all but the last rank column are ready well
                            # before the final rank op; don't let one late
                            # column gate the whole transfer
                            nc.sync.dma_start(out=out_pr[:, lo:lo + 15],
                                              in_=pr_parts[:, lo:lo + 15])
                            nc.sync.dma_start(out=out_pr[:, lo + 15:lo + 16],
                                              in_=pr_parts[:, lo + 15:lo + 16])
                            nc.sync.dma_start(out=out_st[0:1, qs],
                                              in_=st_row[0:1, qs])
                        else:
                            nc.sync.dma_start(out=out_pr[:, lo:lo + 2 * TI],
                                              in_=pr_parts[:, lo:lo + 2 * TI])
                            nc.sync.dma_start(out=out_st[0:1, qs],
                                              in_=st_row[0:1, qs])

                    # ---- PE warmup: throwaway matmuls on ones_t while the
                    # first operand DMAs land, so the PE clock is fully
                    # ramped when the real stream starts.
                    warm = ppb.tile([P, QW], f32, name="warm", tag="st", bufs=1)
                    for w in range(48):
                        nc.tensor.matmul(warm[:, 0:P], lhsT=ones_t, rhs=ones_t,
                                         start=True, stop=True)

                    # ---- phase A: 32 psum quarters [128, 1024] (2 banks,
                    # bufs=3).  Per quarter: 8 matmuls, exp (ACT), rank
                    # max-reduce (DVE), exp-tile accumulate (DVE/Pool split).
                    for jq in range(JQ):
                        for ti in range(TI):
                            if ti == 2 and jq > 0:
                                colsum(jq - 1)
                            if ti == 6:
                                colsum_g0(jq)
                            qg = ti // 4
                            ps = ppa.tile([P, QW], f32, name=f"ps_{ti}_{jq}",
                                          tag="mm", bufs=3)
                            PASSES = ((0, 0, 0), (0, 0, 1), (0, 1, 0),
                                      (0, 1, 1), (1, 0, 0), (1, 0, 1))
                            for j2 in range(QW // 512):
                                for pi, (llo, rlo, cp) in enumerate(PASSES):
                                    ko = (j2 * 2 + cp) * 512
                                    if jq == 0:
                                        ro = 512 + 2560 * rlo + ko
                                        rhs = slk_t[:, :, ro:ro + 512]
                                    else:
                                        rhs = ktq[jq][:, :, 2048 * rlo + ko:
                                                      2048 * rlo + ko + 512]
                                    so = (ti * 2 + cp) * P if ti < 2 else                                         ((ti - 2) * 2 + cp) * P
                                    if ti < 2:
                                        lo_ = 2560 * llo + so
                                        lhsT = slk_t[:, :, lo_:lo_ + P]
                                    else:
                                        lhsT = slr_t[:, :, 1536 * llo + so:
                                                     1536 * llo + so + P]
                                    nc.tensor.matmul(
                                        ps[:, j2 * 512:(j2 + 1) * 512],
                                        lhsT=lhsT,
                                        rhs=rhs,
                                        start=(pi == 0),
                                        stop=(pi == len(PASSES) - 1),
                                        perf_mode=mybir.MatmulPerfMode.DoubleRow,
                                    )
                            # exp(ls*sim) -> colsum accumulator (direct for the
                            # first tile of each group, else scratch + add)
                            if ti % 4 == 0:
                                etgt = acc[qg][jq % 2]
                            else:
                                etgt = pscr.tile([P, QW], bf16, name=f"esc_{ti}_{jq}",
                                                 tag="esc")
                            rks = pscr.tile([P, QW], bf16, name=f"rks_{ti}_{jq}",
                                            tag="rks")
                            dq = dfq[jq]
                            spc = 2 * jq * TI + ti
                            rkc = 2 * jq * TI + TI + ti
                            nc.scalar.activation(
                                etgt, ps, Act.Exp,
                                scale=float(logit_scale),
                                accum_out=pr_parts[:, spc:spc + 1],
                            )
                            last = jq == JQ - 1
                            # on the last quarter's ti6, the accumulate add
                            # goes BEFORE the rank op in the DVE queue so the
                            # final colsum's acc1 input is ready when PE is
                            # rank: rowsum of max(sim, d[j]) off psum.
                            # GPSIMD cannot read PSUM, so this is DVE.
                            nc.vector.scalar_tensor_tensor(
                                out=rks, in0=ps, scalar=0.0, in1=dq,
                                op0=Alu.add, op1=Alu.max,
                                accum_out=pr_parts[:, rkc:rkc + 1],
                            )
                            if last and ti >= TI - 2:
                                esc_tail[ti - (TI - 2)] = etgt
                            elif ti % 4 != 0:
                                fast = ti % 4 == 3
                                eng = nc.vector if fast else nc.gpsimd
                                eng.tensor_add(acc[qg][jq % 2], acc[qg][jq % 2], etgt)
                    colsum(JQ - 1, extra=esc_tail)

    _split_drain_waits(nc)
    return nc


def _l2n(x: np.ndarray) -> np.ndarray:
    n = np.linalg.norm(x, axis=-1, keepdims=True)
    return x / np.maximum(n, 1e-12)


def kernel(emb_point, emb_text, dist_norm, pos_idx, logit_scale):
    global LAST_RESULT
    import os

    ls = float(np.asarray(logit_scale, dtype=np.float64).reshape(-1)[0])
    nc = build_program(ls)

    in_maps = []
    per_sample = []
    for b in range(B):
        refer_n = _l2n(np.asarray(emb_point[b], dtype=np.float32))[np.asarray(pos_idx[b])]
        key_n = _l2n(np.asarray(emb_text[b], dtype=np.float32))
        d = np.einsum("nd,nd->n", refer_n, key_n)          # diag of sim, f32
        d_bf = d.astype(ml_dtypes.bfloat16)
        db_tile = np.ascontiguousarray(
            np.broadcast_to(d_bf[None, :], (P, N)))
        f8 = ml_dtypes.float8_e4m3fn

        def split8(x):
            hi = x.astype(f8)
            lo = (x - hi.astype(np.float32)).astype(f8)
            return hi, lo

        def kpack8(y):
            # [p, s, (jq*4 + j2*2 + cp)*512 + n] = y[jq*1024+j2*512+n,
            #                                        (cp*2+s)*128+p]
            kb = y.reshape(JQ, 2, 512, 2, 2, P)      # jq, j2, n, cp, s, p
            return np.transpose(kb, (5, 4, 0, 3, 1, 2)).reshape(P, 2, -1)

        def spack8(x):
            # [p, s, (ti*2 + cp)*128 + m] = x[ti*128+m, (cp*2+s)*128+p]
            sb = x.reshape(TI, P, 2, 2, P)           # ti, m, cp, s, p
            return np.transpose(sb, (4, 3, 0, 2, 1)).reshape(P, 2, -1)

        ky_hi, ky_lo = split8(key_n)
        kh = kpack8(ky_hi)
        kl = kpack8(ky_lo)
        # per quarter: [hi 2048 | lo 2048]
        ktq_all = np.concatenate(
            [np.concatenate([kh[:, :, jq * 2048:(jq + 1) * 2048],
                             kl[:, :, jq * 2048:(jq + 1) * 2048]], axis=2)
             for jq in range(JQ)], axis=2)
        tx_tile = np.ascontiguousarray(ktq_all[:, :, 4096:])
        per_sample.append((d_bf, d))
        for q in range(QPER):
            slab = refer_n[q * CHUNK:(q + 1) * CHUNK]
            sl_hi, sl_lo = split8(slab)
            sh = spack8(sl_hi)
            sl = spack8(sl_lo)
            slk_tile = np.ascontiguousarray(np.concatenate(
                [sh[:, :, :512], ktq_all[:, :, :2048],
                 sl[:, :, :512], ktq_all[:, :, 2048:4096]], axis=2))
            slr_tile = np.ascontiguousarray(np.concatenate(
                [sh[:, :, 512:], sl[:, :, 512:]], axis=2))
            in_maps.append({"slk": slk_tile, "slr": slr_tile, "tx": tx_tile,
                            "db": db_tile})

    trace = bool(int(os.environ.get("KERNEL_TRACE", "0")))
    res = run_bass_kernel_spmd(nc, in_maps, list(range(8)), trace=trace)
    LAST_RESULT = res

    losses, ranks = [], []
    for b in range(B):
        d_bf, d = per_sample[b]
        d64 = d.astype(np.float64)
        sum_dbf = d_bf.astype(np.float64).sum()
        sp = np.empty(N, np.float64)
        st = np.zeros(N, np.float64)
        rank_b = 0.0
        for q in range(QPER):
            r = res.results[b * QPER + q]
            sl_rows = slice(q * CHUNK, (q + 1) * CHUNK)
            pr = r["out_pr"].astype(np.float64).reshape(P, JQ, 2, TI)
            sp[sl_rows] = pr[:, :, 0, :].sum(axis=1).T.reshape(-1)
            st += r["out_st"].astype(np.float64).reshape(-1)
            # sum_ij max(sim, d_j) - CHUNK * sum_j d_j  (rows of this core)
            rank_b += pr[:, :, 1, :].sum() - CHUNK * sum_dbf
        ce_p = np.log(sp) - ls * d64
        ce_t = np.log(st) - ls * d64
        dn = np.asarray(dist_norm[b], dtype=np.float64)
        losses.append(np.mean(0.5 * (ce_p + ce_t) * dn))
        ranks.append(rank_b)

    contrastive = np.float32(np.mean(losses))
    rank_loss = np.float32(0.5 * np.mean(ranks))
    return contrastive, rank_loss
